# revision 1
# baseline (speedup 1.0000x reference)
"""Trainium2 kernel for nn_ButterflyProduct.

The module applies, 10 times, a weighted (softmax) sum of 10 butterfly
factors to the last dim of x.  Every step is a linear operator on the
1024-dim axis, so the forward pass collapses to one 1024x1024 matrix W
applied to x:  out = x @ W,  W = (M_0 @ ... @ M_9)^T,
M_i = sum_j softmax(logit)[i,j] * B_j.

W is composed on the host (float64), and the batch application runs
data-parallel on 8 cores: each core multiplies its [1024,1024] x-shard
by W.  All wire traffic is narrow (bf16 or fp8): 6 MiB/core (bf16
variant) or 4+2 MiB (fp8x3) vs 12.6 MB for the fp32 version.

Variants (VARIANT):
  "bf16"  - single bf16 matmul.  PE cost 65536 cycles/core.
  "fp8x3" - three fp8e4m3 DoubleRow matmuls (hi*hi + hi*lo + lo*hi with
            hi/lo a two-level fp8 decomposition at one common scale so
            all terms share one PSUM accumulator).  PE cost 49152
            cycles/core; rel err ~3.7e-3 (simulated, deterministic).

Device schedule (per core):
  - x arrives host-transposed and pre-packed, so no on-device transpose
    and every DMA descriptor is a contiguous 2-4KB row.
  - stationary = x k-subtile  [128(k), 128(batch)]
    moving     = W k-chunk    [128(k), 512(n)]
    psum acc   = out rows     [128(batch), 512(n)]   (natural layout)
  - phase A (batch blocks 0-3, 8 accs = 8 PSUM banks) consumes (W,x)
    chunks in arrival (wavefront) order while they stream in; phase B
    (blocks 4-7) runs from SBUF-resident data.
  - PSUM -> SBUF evacuation casts to bf16, alternating DVE / ACT.
  - a few warmup matmuls on a memset tile ramp the PE p-state while the
    first chunks are still in flight.
"""

import numpy as np
import ml_dtypes
from contextlib import ExitStack

import concourse.bass as bass
import concourse.bacc as bacc
import concourse.mybir as mybir
import concourse.tile as tile
from concourse.bass_utils import run_bass_kernel_spmd

SIZE = 1024
M = 10
N_TERMS = 10
BATCH = 8192
NCORES = 8
SHARD = BATCH // NCORES  # 1024
DIAGS = [1 << (M - 1 - j) for j in range(M)]

P = 128
NB = SHARD // P          # 8 batch row-blocks per core
NK = SIZE // P           # 8 contraction tiles
NT = SIZE // (2 * P)     # 4 double-row k-pairs (fp8)
NFREE = 512              # psum bank free size (fp32)
NN = SIZE // NFREE       # 2 output column chunks
PHASE = 4                # batch blocks per psum phase

VARIANT = "bf16"         # "bf16" | "fp8x3"
WU_N = 6                 # warmup matmuls
FP8_SCALE = 16.0         # power of two; host rescales output by 1/S^2

BF16 = ml_dtypes.bfloat16
F8 = ml_dtypes.float8_e4m3


def _compose_w(diag, subpad, suppad, logit):
    """Compose the full linear operator W (float64) so out = x @ W."""
    lg = logit.astype(np.float64)
    e = np.exp(lg - lg.max(axis=-1, keepdims=True))
    prob = e / e.sum(axis=-1, keepdims=True)          # (N_TERMS, M)
    dg = diag.astype(np.float64)
    sb = subpad.astype(np.float64)
    sp = suppad.astype(np.float64)

    A = np.eye(SIZE, dtype=np.float64)
    for i in range(N_TERMS)[::-1]:
        D = (prob[i][:, None] * dg).sum(0)            # combined diagonal
        out = D[:, None] * A
        for j in range(M):
            d = DIAGS[j]
            out[d:] += (prob[i, j] * sb[j, d:])[:, None] * A[:-d]
            out[:-d] += (prob[i, j] * sp[j, :-d])[:, None] * A[d:]
        A = out                                       # A = M_i @ ... @ M_9
    return np.ascontiguousarray(A.T.astype(np.float32))


def _slim_drain_and_barrier(self, tick_clock, wait_clock):
    """Replacement for TileContext._drain_and_barrier: keep the sync-engine
    drain that waits for every queue/engine tick (this is what guarantees the
    output DMAs have landed), drop the two all-engine barriers and the
    semaphore clears -- the Bass preamble re-clears all semaphores at the next
    execution's start, so end-of-kernel hygiene costs ~7us for nothing."""
    from concourse.tile import ScopedClock

    drain_inst = self.nc.sync.drain()
    wait_clock.add_sem_waits(
        drain_inst.ins, ScopedClock({None: tick_clock.global_clock})
    )
    popped = self.nc._tile_sem_poison_stack.pop()
    assert popped is self._sem_poison


def _phase_pairs(n_chunks, avail_w, avail_x, jlo, jhi):
    """(k_chunk, j) pairs for batch blocks [jlo, jhi), sorted by the arrival
    position of their inputs (wavefront order), tie-broken k-ascending so
    each accumulator sees k=0 first and k=last last."""
    pairs = []
    for k in range(n_chunks):
        for j in range(jlo, jhi):
            pairs.append((max(avail_w[k], avail_x[j]), k, j))
    pairs.sort()
    return [(k, j) for _, k, j in pairs]


def _build_program(variant):
    nc = bacc.Bacc(None, target_bir_lowering=False)
    f32 = mybir.dt.float32
    dt = mybir.dt.bfloat16 if variant == "bf16" else mybir.dt.float8e4

    if variant == "bf16":
        # [m, p, 0:1024] = W[128m+p, :], [m, p, 1024+k*128+b] = x^T[128k+p, 128m+b]
        in_all = nc.dram_tensor("in_all", [NK, P, 2048], dt, kind="ExternalInput")
        w_in = x_in = None
    else:
        # w_in[t, p, i*1024+n] = Whi[256t+128i+p, n] ; +2048 offset for Wlo
        w_in = nc.dram_tensor("w_in", [NT, P, 4096], dt, kind="ExternalInput")
        # x_in[j, p, t*256+i*128+b] = xThi[256t+128i+p, 128j+b]; +1024 for xTlo
        x_in = nc.dram_tensor("x_in", [NB, P, 2048], dt, kind="ExternalInput")
        in_all = None
    out_d = nc.dram_tensor("out", [SHARD, SIZE], mybir.dt.bfloat16,
                           kind="ExternalOutput")

    orig_dab = tile.TileContext._drain_and_barrier
    tile.TileContext._drain_and_barrier = _slim_drain_and_barrier
    try:
        _emit_body(nc, variant, in_all, w_in, x_in, out_d)
    finally:
        tile.TileContext._drain_and_barrier = orig_dab

    nc.finalize()
    return nc


def _emit_body(nc, variant, in_all, w_in, x_in, out_d):
    f32 = mybir.dt.float32
    dt = mybir.dt.bfloat16 if variant == "bf16" else mybir.dt.float8e4

    with ExitStack() as ctx:
        tc = ctx.enter_context(tile.TileContext(nc))
        const = ctx.enter_context(tc.tile_pool(name="const", bufs=1))
        inpool = ctx.enter_context(tc.tile_pool(name="inpool", bufs=1))
        opool = ctx.enter_context(tc.tile_pool(name="opool", bufs=NB))
        psum = ctx.enter_context(tc.tile_pool(name="psum", bufs=8, space="PSUM"))

        # -- warmup: ramp the PE p-state while the first chunks stream in.
        # memset on DVE (free ~1.2us before gpsimd clears its preamble work)
        wu = const.tile([P, NFREE], dt)
        nc.vector.memset(wu[:], 1.0)
        wups = psum.tile([P, NFREE], f32, tag="ps", name="wups")
        for _ in range(WU_N):
            nc.tensor.matmul(wups[:], wu[:, :P], wu[:], start=True, stop=True)

        # -- inbound DMAs + slice helpers
        if variant == "bf16":
            # x is packed k-major (chunk m carries x^T rows for k-tile m,
            # ALL batch blocks), so every arriving chunk immediately feeds
            # 8 phase-A matmuls.  chunk 0 is host-packed as
            # [w0h0 | x(k0,b0:512) | w0h1 | x(k0,b512:)] and split in two
            # DMAs so the first matmuls' inputs land one half-chunk early.
            in_sb = inpool.tile([P, NK * 2048], dt, tag="in")
            nc.sync.dma_start(in_sb[:, 0:1024], in_all[0, :, 0:1024])
            nc.sync.dma_start(in_sb[:, 1024:2048], in_all[0, :, 1024:2048])
            for m in range(1, NK):
                nc.sync.dma_start(
                    in_sb[:, m * 2048:(m + 1) * 2048], in_all[m, :, :])
            n_chunks = NK

            def w_pos(k, h):
                return h if k == 0 else k + 1

            def x_pos(j, k):
                if k == 0:
                    return 0 if j < PHASE else 1
                return k + 1

            def rhs_sl(k, h):            # W chunk k, col half h  [128, 512]
                if k == 0:
                    return in_sb[:, h * 1024:h * 1024 + NFREE]
                return in_sb[:, k * 2048 + h * NFREE:k * 2048 + (h + 1) * NFREE]

            def lhs_sl_bf(k, j):         # x^T k-subtile for batch block j
                if k == 0:
                    o = NFREE + j * P if j < PHASE else 1536 + (j - PHASE) * P
                else:
                    o = k * 2048 + 1024 + j * P
                return in_sb[:, o:o + P]
        else:
            w_sb = inpool.tile([P, NT * 4096], dt, tag="w")
            x_sb = inpool.tile([P, NB * 2048], dt, tag="x")
            # issue order: w0 x0 w1 x1 w2 x2 w3 x3 x4..x7
            avail_w, avail_x, pos = [0] * NT, [0] * NB, 0
            for t in range(NT):
                nc.sync.dma_start(w_sb[:, t * 4096:(t + 1) * 4096], w_in[t, :, :])
                avail_w[t] = pos
                pos += 1
                nc.sync.dma_start(x_sb[:, t * 2048:(t + 1) * 2048], x_in[t, :, :])
                avail_x[t] = pos
                pos += 1
            for j in range(NT, NB):
                nc.sync.dma_start(x_sb[:, j * 2048:(j + 1) * 2048], x_in[j, :, :])
                avail_x[j] = pos
                pos += 1
            n_chunks = NT

            def rhs_sl(t, h, lo=0):      # W chunk t (hi/lo), col half h [128,2,512]
                base = t * 4096 + lo * 2048
                v = w_sb[:, base:base + 2048].rearrange("p (i n) -> p i n", i=2)
                return v[:, :, h * NFREE:(h + 1) * NFREE]

            def lhs_sl(t, j, lo=0):      # x^T pair-subtile (hi/lo)  [128, 2, 128]
                base = j * 2048 + lo * 1024 + t * 256
                return x_sb[:, base:base + 256].rearrange("p (i b) -> p i b", i=2)

        # -- matmul phases
        DR = mybir.MatmulPerfMode.DoubleRow
        o_tiles = {}

        def run_phase(jlo, jhi):
            accs = {}
            for j in range(jlo, jhi):
                for h in range(NN):
                    accs[(j, h)] = psum.tile([P, NFREE], f32, tag="ps",
                                             name=f"acc_{j}_{h}")
            if variant == "bf16":
                trips = sorted(
                    (max(w_pos(k, h), x_pos(j, k)), k, j, h)
                    for k in range(n_chunks)
                    for j in range(jlo, jhi) for h in range(NN))
                for _, k, j, h in trips:
                    nc.tensor.matmul(accs[(j, h)][:], lhs_sl_bf(k, j),
                                     rhs_sl(k, h),
                                     start=k == 0, stop=k == n_chunks - 1)
            else:
                for k, j in _phase_pairs(n_chunks, avail_w, avail_x, jlo, jhi):
                    first, last = k == 0, k == n_chunks - 1
                    # terms: hi*hi, hi*lo, lo*hi -- same scale, one accumulator
                    terms = [(0, 0), (0, 1), (1, 0)]
                    for ti, (xlo, wlo) in enumerate(terms):
                        lhs = lhs_sl(k, j, lo=xlo)
                        for h in range(NN):
                            nc.tensor.matmul(
                                accs[(j, h)][:], lhs, rhs_sl(k, h, lo=wlo),
                                start=first and ti == 0,
                                stop=last and ti == len(terms) - 1,
                                perf_mode=DR)
            # evacuate: cast fp32 psum -> bf16 SBUF, DVE (h0) / ACT (h1) in
            # parallel; each half's out-DMA issues from SP (idle after the
            # in-stream) as soon as that half's copy lands.
            for j in range(jlo, jhi):
                ot = opool.tile([P, SIZE], mybir.dt.bfloat16, tag="ot",
                                name=f"ot_{j}")
                nc.vector.tensor_copy(ot[:, :NFREE], accs[(j, 0)][:])
                nc.scalar.copy(ot[:, NFREE:], accs[(j, 1)][:])
                o_tiles[j] = ot
                nc.sync.dma_start(
                    out_d[j * P:(j + 1) * P, 0:NFREE], ot[:, :NFREE])
                nc.sync.dma_start(
                    out_d[j * P:(j + 1) * P, NFREE:], ot[:, NFREE:])

        run_phase(0, PHASE)
        run_phase(PHASE, NB)


_progs = {}


def _get_prog(variant):
    if variant not in _progs:
        _progs[variant] = _build_program(variant)
    return _progs[variant]


def _pack_inputs(x, W, variant):
    """Per-core host-side packing into DMA-optimal layouts."""
    in_maps = []
    if variant == "bf16":
        w_arr = W.astype(BF16).reshape(NK, P, SIZE)
        for c in range(NCORES):
            xs = x[c * SHARD:(c + 1) * SHARD]
            # x_arr[m, p, b] = x^T[128m+p, b] = xs[b, 128m+p]  (k-major)
            x_arr = np.ascontiguousarray(xs.T).astype(BF16).reshape(
                NK, P, SIZE)
            in_all = np.ascontiguousarray(
                np.concatenate([w_arr, x_arr], axis=-1))
            # chunk 0 reorder: [w0h0 | x(k0,b0:512) | w0h1 | x(k0,b512:)]
            # so the first matmuls' inputs ride the head DMA
            in_all[0] = np.concatenate(
                [w_arr[0][:, :NFREE], x_arr[0][:, :NFREE],
                 w_arr[0][:, NFREE:], x_arr[0][:, NFREE:]], axis=-1)
            in_maps.append({"in_all": in_all})
    else:
        S = FP8_SCALE
        Ws = W.astype(np.float64) * S
        Wh = (Ws).astype(np.float32).astype(F8)
        Wl = (Ws - Wh.astype(np.float64)).astype(np.float32).astype(F8)

        def arrange_w(A):  # [1024, 1024] -> [t, p, i*1024+n]
            return np.ascontiguousarray(
                A.reshape(NT, 2, P, SIZE).transpose(0, 2, 1, 3)
            ).reshape(NT, P, 2 * SIZE)
        w_in = np.ascontiguousarray(
            np.concatenate([arrange_w(Wh), arrange_w(Wl)], axis=-1))

        for c in range(NCORES):
            xs = x[c * SHARD:(c + 1) * SHARD].astype(np.float64) * S
            xt = np.ascontiguousarray(xs.T)               # [size, batch]
            xh = xt.astype(np.float32).astype(F8)
            xl = (xt - xh.astype(np.float64)).astype(np.float32).astype(F8)

            def arrange_x(A):  # [1024 size, 1024 batch] -> [j, p, t*256+i*128+b]
                return np.ascontiguousarray(
                    A.reshape(NT, 2, P, NB, P).transpose(3, 2, 0, 1, 4)
                ).reshape(NB, P, SIZE)
            x_in = np.ascontiguousarray(
                np.concatenate([arrange_x(xh), arrange_x(xl)], axis=-1))
            in_maps.append({"w_in": w_in, "x_in": x_in})
    return in_maps


def kernel(x, diag, subpad, suppad, logit):
    W = _compose_w(np.asarray(diag), np.asarray(subpad),
                   np.asarray(suppad), np.asarray(logit))
    x = np.ascontiguousarray(np.asarray(x, dtype=np.float32))
    prog = _get_prog(VARIANT)
    in_maps = _pack_inputs(x, W, VARIANT)
    res = run_bass_kernel_spmd(prog, in_maps, list(range(NCORES)))
    outs = [r["out"].astype(np.float32) for r in res.results]
    out = np.concatenate(outs, axis=0)
    if VARIANT == "fp8x3":
        out /= FP8_SCALE * FP8_SCALE
    return np.ascontiguousarray(out)



# revision 2
# speedup vs baseline: 1.1568x; 1.1568x over previous
"""Trainium2 kernel for nn_ButterflyProduct.

The module applies, 10 times, a weighted (softmax) sum of 10 butterfly
factors to the last dim of x.  Every step is a linear operator on the
1024-dim axis, so the forward pass collapses to one 1024x1024 matrix W
applied to x:  out = x @ W,  W = (M_0 @ ... @ M_9)^T,
M_i = sum_j softmax(logit)[i,j] * B_j.

W is composed on the host (float64), and the batch application runs
data-parallel on 8 cores: each core multiplies its [1024,1024] x-shard
by W as a single bf16 pass (128 matmuls of [128k,128b]x[128k,512n]).

Schedule (per core), tuned against the NTFF trace:
  - x arrives host-transposed and pre-packed with W in DMA-arrival
    order; the head pieces are small so the first real matmul starts
    ~3us earlier than a uniform-chunk stream.
  - warmup matmuls (N=256 on a memset tile) bridge the PE from the
    preamble barrier to first-data with no idle gap, so the HAM clock
    ramp completes during the DMA latency instead of after it.
  - stationary = x k-subtile [128(k), 128(batch)]
    moving     = W k-chunk   [128(k), 512(n)]
    psum acc   = out rows    [128(batch), 512(n)]
  - phase A (batch blocks 0-3, 8 accs = 8 psum banks) consumes (W,x)
    pieces in arrival (wavefront) order; phase B (blocks 4-7) runs
    acc-major from SBUF-resident data so accumulators complete (and
    evacuate) staggered instead of bunched at the end.
  - each acc is evacuated the moment its k-loop stops: fp32 psum ->
    bf16 SBUF cast alternating DVE / ACT, out-DMA alternating the two
    HWDGE queues (sync / scalar).  The final acc is split in half
    across both cast engines and both DMA queues to minimize the tail.
"""

import numpy as np
import ml_dtypes
from contextlib import ExitStack

import concourse.bass as bass
import concourse.bacc as bacc
import concourse.mybir as mybir
import concourse.tile as tile
from concourse.bass_utils import run_bass_kernel_spmd

SIZE = 1024
M = 10
N_TERMS = 10
BATCH = 8192
NCORES = 8
SHARD = BATCH // NCORES  # 1024
DIAGS = [1 << (M - 1 - j) for j in range(M)]

P = 128
NB = SHARD // P          # 8 batch row-blocks per core
NK = SIZE // P           # 8 contraction tiles
NFREE = 512              # psum bank free size (fp32)
NN = SIZE // NFREE       # 2 output column chunks
PHASE = 4                # batch blocks in the arrival-paced phase

VARIANT = "bf16"
WU_N = 10                # warmup matmuls (N=256) bridging to first data

BF16 = ml_dtypes.bfloat16

# per-k column layout of the packed inA tensor:
#   k == 0 : [ W h0 (512) | xA (512) | W h1 (512) ]   (head split)
#   k >= 1 : [ W (1024)   | xA (512) ]
KW = 3 * NFREE           # 1536 cols per k-chunk in inA


def _compose_w(diag, subpad, suppad, logit):
    """Compose the full linear operator W (float64) so out = x @ W."""
    lg = logit.astype(np.float64)
    e = np.exp(lg - lg.max(axis=-1, keepdims=True))
    prob = e / e.sum(axis=-1, keepdims=True)          # (N_TERMS, M)
    dg = diag.astype(np.float64)
    sb = subpad.astype(np.float64)
    sp = suppad.astype(np.float64)

    A = np.eye(SIZE, dtype=np.float64)
    for i in range(N_TERMS)[::-1]:
        D = (prob[i][:, None] * dg).sum(0)            # combined diagonal
        out = D[:, None] * A
        for j in range(M):
            d = DIAGS[j]
            out[d:] += (prob[i, j] * sb[j, d:])[:, None] * A[:-d]
            out[:-d] += (prob[i, j] * sp[j, :-d])[:, None] * A[d:]
        A = out                                       # A = M_i @ ... @ M_9
    return np.ascontiguousarray(A.T.astype(np.float32))


def _slim_drain_and_barrier(self, tick_clock, wait_clock):
    """Replacement for TileContext._drain_and_barrier: keep the sync-engine
    drain that waits for every queue/engine tick (this is what guarantees the
    output DMAs have landed), drop the two all-engine barriers and the
    semaphore clears -- the compiler postamble re-clears all semaphores
    anyway, so end-of-kernel hygiene costs ~7us for nothing."""
    from concourse.tile import ScopedClock

    drain_inst = self.nc.sync.drain()
    wait_clock.add_sem_waits(
        drain_inst.ins, ScopedClock({None: tick_clock.global_clock})
    )
    popped = self.nc._tile_sem_poison_stack.pop()
    assert popped is self._sem_poison


def _build_program(variant):
    nc = bacc.Bacc(None, target_bir_lowering=False)
    dt = mybir.dt.bfloat16

    # inA[p, k*KW + c]: W chunks + x^T (batch blocks 0-3), arrival order.
    in_a = nc.dram_tensor("in_a", [P, NK * KW], dt, kind="ExternalInput")
    # inB[p, k*512 + c]: x^T batch blocks 4-7, k-major.
    in_b = nc.dram_tensor("in_b", [P, NK * NFREE], dt, kind="ExternalInput")
    out_d = nc.dram_tensor("out", [SHARD, SIZE], mybir.dt.bfloat16,
                           kind="ExternalOutput")

    orig_dab = tile.TileContext._drain_and_barrier
    tile.TileContext._drain_and_barrier = _slim_drain_and_barrier
    try:
        _emit_body(nc, in_a, in_b, out_d)
    finally:
        tile.TileContext._drain_and_barrier = orig_dab

    nc.finalize()
    return nc


def _emit_body(nc, in_a, in_b, out_d):
    f32 = mybir.dt.float32
    dt = mybir.dt.bfloat16

    with ExitStack() as ctx:
        tc = ctx.enter_context(tile.TileContext(nc))
        const = ctx.enter_context(tc.tile_pool(name="const", bufs=1))
        inpool = ctx.enter_context(tc.tile_pool(name="inpool", bufs=1))
        opool = ctx.enter_context(tc.tile_pool(name="opool", bufs=2 * NB))
        psum = ctx.enter_context(tc.tile_pool(name="psum", bufs=8, space="PSUM"))

        a_sb = inpool.tile([P, NK * KW], dt, tag="ina")
        b_sb = inpool.tile([P, NK * NFREE], dt, tag="inb")

        # -- inbound DMAs, issue order == arrival order (sync HWDGE queue).
        nc.sync.dma_start(a_sb[:, 0:1024], in_a[:, 0:1024])          # Wk0h0+xA0
        nc.sync.dma_start(a_sb[:, 1024:KW], in_a[:, 1024:KW])       # Wk0h1
        for k in range(1, NK):
            nc.sync.dma_start(a_sb[:, k * KW:(k + 1) * KW],
                              in_a[:, k * KW:(k + 1) * KW])
        half_b = NK * NFREE // 2
        nc.sync.dma_start(b_sb[:, 0:half_b], in_b[:, 0:half_b])
        nc.sync.dma_start(b_sb[:, half_b:], in_b[:, half_b:])

        # -- warmup: keep the PE busy from the preamble barrier until the
        # first data piece lands, so the HAM ramp completes during DMA
        # latency.  DVE memset is the first user op on the vector queue.
        wu = const.tile([P, 256], dt)
        nc.vector.memset(wu[:], 1.0)
        wups = psum.tile([P, 256], f32, tag="ps", name="wups")
        for _ in range(WU_N):
            nc.tensor.matmul(wups[:], wu[:, :P], wu[:], start=True, stop=True)

        def rhs_sl(k, h):            # W chunk k, col half h  [128, 512]
            if k == 0:
                o = 0 if h == 0 else 1024
            else:
                o = k * KW + h * NFREE
            return a_sb[:, o:o + NFREE]

        def lhs_sl(k, j):            # x^T k-subtile for batch block j
            if j < PHASE:
                o = (512 if k == 0 else k * KW + 1024) + j * P
                return a_sb[:, o:o + P]
            o = k * NFREE + (j - PHASE) * P
            return b_sb[:, o:o + P]

        accs = {}
        n_evac = [0]

        def evac(j, h, last=False):
            acc = accs[(j, h)]
            ot = opool.tile([P, NFREE], dt, tag="ot", name=f"ot_{j}_{h}")
            dst = out_d[j * P:(j + 1) * P, h * NFREE:(h + 1) * NFREE]
            if last:
                nc.vector.tensor_copy(ot[:, :256], acc[:, :256])
                nc.scalar.copy(ot[:, 256:], acc[:, 256:])
                nc.sync.dma_start(dst[:, :256], ot[:, :256])
                nc.scalar.dma_start(dst[:, 256:], ot[:, 256:])
            else:
                a = n_evac[0]
                if a % 2 == 0:
                    nc.vector.tensor_copy(ot[:], acc[:])
                    nc.sync.dma_start(dst, ot[:])
                else:
                    nc.scalar.copy(ot[:], acc[:])
                    nc.scalar.dma_start(dst, ot[:])
            n_evac[0] += 1

        # -- phase A (j 0..3): arrival-ordered trips.
        for j in range(PHASE):
            for h in range(NN):
                accs[(j, h)] = psum.tile([P, NFREE], f32, tag="ps",
                                         name=f"acc_{j}_{h}")

        def w_pos(k, h):
            return h if k == 0 else k + 1

        def x_pos(k):
            return 0 if k == 0 else k + 1

        trips = sorted(
            (max(w_pos(k, h), x_pos(k)), k, j, h)
            for k in range(NK) for j in range(PHASE) for h in range(NN))
        for _, k, j, h in trips:
            nc.tensor.matmul(accs[(j, h)][:], lhs_sl(k, j), rhs_sl(k, h),
                             start=k == 0, stop=k == NK - 1)
            if k == NK - 1:
                evac(j, h)

        # -- phase B (j 4..7): acc-major from SBUF-resident data, so each
        # acc completes (and evacuates) as early as possible.
        for j in range(PHASE, NB):
            for h in range(NN):
                accs[(j, h)] = psum.tile([P, NFREE], f32, tag="ps",
                                         name=f"acc_{j}_{h}")
                for k in range(NK):
                    nc.tensor.matmul(accs[(j, h)][:], lhs_sl(k, j),
                                     rhs_sl(k, h),
                                     start=k == 0, stop=k == NK - 1)
                evac(j, h, last=(j == NB - 1 and h == NN - 1))


_progs = {}


def _get_prog(variant):
    if variant not in _progs:
        _progs[variant] = _build_program(variant)
    return _progs[variant]


def _pack_inputs(x, W, variant):
    """Per-core host-side packing into DMA-arrival layouts."""
    w16 = W.astype(BF16)                              # [1024 k, 1024 n]
    in_maps = []
    for c in range(NCORES):
        xs = x[c * SHARD:(c + 1) * SHARD]
        xt = np.ascontiguousarray(xs.T).astype(BF16)  # [1024 k, 1024 b]
        in_a = np.empty((P, NK * KW), dtype=BF16)
        in_b = np.empty((P, NK * NFREE), dtype=BF16)
        for k in range(NK):
            wk = w16[k * P:(k + 1) * P]               # [128, 1024]
            xk = xt[k * P:(k + 1) * P]                # [128, 1024]
            col = k * KW
            if k == 0:
                in_a[:, col:col + 512] = wk[:, :512]
                in_a[:, col + 512:col + 1024] = xk[:, :512]
                in_a[:, col + 1024:col + KW] = wk[:, 512:]
            else:
                in_a[:, col:col + 1024] = wk
                in_a[:, col + 1024:col + KW] = xk[:, :512]
            in_b[:, k * NFREE:(k + 1) * NFREE] = xk[:, 512:]
        in_maps.append({"in_a": in_a, "in_b": in_b})
    return in_maps


def kernel(x, diag, subpad, suppad, logit):
    W = _compose_w(np.asarray(diag), np.asarray(subpad),
                   np.asarray(suppad), np.asarray(logit))
    x = np.ascontiguousarray(np.asarray(x, dtype=np.float32))
    prog = _get_prog(VARIANT)
    in_maps = _pack_inputs(x, W, VARIANT)
    res = run_bass_kernel_spmd(prog, in_maps, list(range(NCORES)))
    outs = [r["out"].astype(np.float32) for r in res.results]
    return np.ascontiguousarray(np.concatenate(outs, axis=0))
